# revision 1
# baseline (speedup 1.0000x reference)
"""GQA kernel for Trainium2, 8 NeuronCores.

Problem: B=2, T=2048, D=2048, 16 query heads / 2 KV heads, d_head=128, causal.

Sharding: core c -> batch b = c//4, head-quarter q = c%4 (query heads
4q..4q+3, kv head q//2). Each core computes its 4 heads' attention and a
partial output projection (its Wo rows); host sums the 4 partials per batch
and adds bo.

Host marshalling: weights and x are pre-cast to bf16 (same rounding the
kernel would do on-chip) and x is supplied transposed (xT = x[b].T), which
is the layout every projection matmul consumes.

On-core dataflow (bf16 matmuls, fp32 PSUM accum), interleaved in 4 rounds
over 512-wide t-slices so PE stays continuously fed:
  round j: project KT/QT/VT for slice j; PE-transpose VT -> V natural;
           attention (h, j) for all 4 heads over tk blocks 0..4j+3
           (S_T tiles [tk,tq]; exp on ACT; causal zeroing of the diagonal
           blocks on GpSimd post-exp; OT accum on PE; row-sum accum on DVE
           with one fp32 ones-matmul per (h,j) for the partition reduction);
           output projection for the 4 t-tiles of slice j.
Engine budget: PE ~matmuls only, ACT ~exp+proj epilogues, DVE ~copies+
row-sum+normalize, GpSimd ~causal masks, 4 DMA queues for input streaming.
"""

import numpy as np
import ml_dtypes
from contextlib import ExitStack

import concourse.bass as bass
from concourse import bacc
import concourse.mybir as mybir
import concourse.tile as tile
from concourse.bass_utils import run_bass_kernel_spmd
from concourse.masks import make_identity

F32 = mybir.dt.float32
BF16 = mybir.dt.bfloat16

D = 2048
T = 2048
DH = 128
B = 2
HPC = 4            # query heads per core
NCORES = 8
SCALE = 1.0 / float(np.sqrt(128.0))

_CACHE = {}


def _build_nc():
    nc = bacc.Bacc("TRN2", target_bir_lowering=False, debug=False,
                   num_devices=NCORES)

    xt = nc.dram_tensor("xt", [D, T], BF16, kind="ExternalInput")
    wq = nc.dram_tensor("wq", [D, HPC * DH], BF16, kind="ExternalInput")
    wk = nc.dram_tensor("wk", [D, DH], BF16, kind="ExternalInput")
    wv = nc.dram_tensor("wv", [D, DH], BF16, kind="ExternalInput")
    wo = nc.dram_tensor("wo", [HPC * DH, D], BF16, kind="ExternalInput")
    bqm = nc.dram_tensor("bqm", [DH, HPC], F32, kind="ExternalInput")
    bkm = nc.dram_tensor("bkm", [DH, 1], F32, kind="ExternalInput")
    bvm = nc.dram_tensor("bvm", [DH, 1], F32, kind="ExternalInput")
    part = nc.dram_tensor("part", [T, D], F32, kind="ExternalOutput")

    with ExitStack() as ctx:
        tc = ctx.enter_context(tile.TileContext(nc))
        persist = ctx.enter_context(tc.tile_pool(name="persist", bufs=1))
        work = ctx.enter_context(tc.tile_pool(name="work", bufs=3))
        psum = ctx.enter_context(tc.tile_pool(name="psum", bufs=2, space="PSUM"))

        # ---- constants ----
        ones32 = persist.tile([128, 128], F32, tag="ones32", name="ones32")
        nc.vector.memset(ones32, 1.0)
        ident = persist.tile([128, 128], BF16, tag="ident", name="ident")
        make_identity(nc, ident)

        bq_sb = persist.tile([DH, HPC], F32, tag="bq", name="bq_sb")
        nc.sync.dma_start(out=bq_sb, in_=bqm[:, :])
        bk_sb = persist.tile([DH, 1], F32, tag="bk", name="bk_sb")
        nc.sync.dma_start(out=bk_sb, in_=bkm[:, :])
        bv_sb = persist.tile([DH, 1], F32, tag="bv", name="bv_sb")
        nc.sync.dma_start(out=bv_sb, in_=bvm[:, :])

        # ---- inputs -> SBUF (already bf16), streamed on 4 DMA queues in
        # consumption order: wk, xT slice 0, wq, wv, xT slices 1-3, wo ----
        queues = [nc.sync, nc.scalar, nc.gpsimd]
        _qi = [0]

        def dma(out, in_):
            queues[_qi[0] % 3].dma_start(out=out, in_=in_)
            _qi[0] += 1

        xT = [persist.tile([128, T], BF16, tag=f"xT{kb}", name=f"xT{kb}")
              for kb in range(16)]
        wq_sb = []
        wk_sb = []
        wv_sb = []
        for kb in range(16):
            wkt = persist.tile([128, 128], BF16, tag=f"wk{kb}", name=f"wk_sb{kb}")
            dma(wkt, wk[kb * 128:(kb + 1) * 128, :])
            wk_sb.append(wkt)
        for kb in range(16):
            dma(xT[kb][:, 0:512], xt[kb * 128:(kb + 1) * 128, 0:512])
        for kb in range(16):
            wqt = persist.tile([128, 512], BF16, tag=f"wq{kb}", name=f"wq_sb{kb}")
            dma(wqt, wq[kb * 128:(kb + 1) * 128, :])
            wq_sb.append(wqt)
        for kb in range(16):
            wvt = persist.tile([128, 128], BF16, tag=f"wv{kb}", name=f"wv_sb{kb}")
            dma(wvt, wv[kb * 128:(kb + 1) * 128, :])
            wv_sb.append(wvt)
        for js in range(1, 4):
            for kb in range(16):
                dma(xT[kb][:, js * 512:(js + 1) * 512],
                    xt[kb * 128:(kb + 1) * 128, js * 512:(js + 1) * 512])
        wo_sb = []
        for h in range(HPC):
            wot = persist.tile([128, D], BF16, tag=f"wo{h}", name=f"wo_sb{h}")
            dma(wot, wo[h * 128:(h + 1) * 128, :])
            wo_sb.append(wot)

        # ---- persistent activations ----
        qT = [persist.tile([128, T], BF16, tag=f"qT{h}", name=f"qT{h}")
              for h in range(HPC)]
        kT = persist.tile([128, T], BF16, tag="kT", name="kT")
        v_sb = [persist.tile([128, DH], BF16, tag=f"v{t}", name=f"v{t}")
                for t in range(16)]
        oT = [persist.tile([128, T], BF16, tag=f"oT{h}", name=f"oT{h}")
              for h in range(HPC)]

        for j in range(4):
            sl = slice(j * 512, (j + 1) * 512)

            # --- projections for t-slice j ---
            kps = psum.tile([128, 512], F32, tag="acc", bufs=3, name=f"kps{j}")
            for kb in range(16):
                nc.tensor.matmul(out=kps, lhsT=wk_sb[kb], rhs=xT[kb][:, sl],
                                 start=(kb == 0), stop=(kb == 15))
            nc.scalar.activation(out=kT[:, sl], in_=kps,
                                 func=mybir.ActivationFunctionType.Identity,
                                 bias=bk_sb[:, 0:1], scale=1.0)

            for h in range(HPC):
                qps = psum.tile([128, 512], F32, tag="acc", bufs=3,
                                name=f"qps{j}_{h}")
                for kb in range(16):
                    nc.tensor.matmul(out=qps,
                                     lhsT=wq_sb[kb][:, h * 128:(h + 1) * 128],
                                     rhs=xT[kb][:, sl],
                                     start=(kb == 0), stop=(kb == 15))
                nc.scalar.activation(out=qT[h][:, sl], in_=qps,
                                     func=mybir.ActivationFunctionType.Identity,
                                     bias=bq_sb[:, h:h + 1], scale=1.0)

            # VT projection for slice j, then PE-transpose to natural V
            vps = psum.tile([128, 512], F32, tag="acc", bufs=3, name=f"vps{j}")
            for kb in range(16):
                nc.tensor.matmul(out=vps, lhsT=wv_sb[kb], rhs=xT[kb][:, sl],
                                 start=(kb == 0), stop=(kb == 15))
            vt_sb = work.tile([128, 512], BF16, tag="vt", bufs=2,
                              name=f"vt{j}")
            nc.scalar.activation(out=vt_sb, in_=vps,
                                 func=mybir.ActivationFunctionType.Identity,
                                 bias=bv_sb[:, 0:1], scale=1.0)
            vtp = psum.tile([128, 512], BF16, tag="op", bufs=2, name=f"vtp{j}")
            for sub in range(4):
                nc.tensor.transpose(vtp[:, sub * 128:(sub + 1) * 128],
                                    vt_sb[:, sub * 128:(sub + 1) * 128],
                                    ident)
            for sub in range(4):
                nc.vector.tensor_copy(out=v_sb[4 * j + sub],
                                      in_=vtp[:, sub * 128:(sub + 1) * 128])

            # --- attention for all heads, tq-slice j ---
            ntk = 4 * (j + 1)
            for h in range(HPC):
                otps = psum.tile([128, 512], F32, tag="acc", bufs=3,
                                 name=f"otps{h}_{j}")
                racc = work.tile([128, 512], F32, tag="racc", bufs=2,
                                 name=f"racc{h}_{j}")
                for tkb in range(ntk):
                    sps = psum.tile([128, 512], F32, tag="sp", bufs=3,
                                    name=f"sps{h}_{j}_{tkb}")
                    nc.tensor.matmul(out=sps,
                                     lhsT=kT[:, tkb * 128:(tkb + 1) * 128],
                                     rhs=qT[h][:, sl],
                                     start=True, stop=True)
                    pt = work.tile([128, 512], BF16, tag="pt", bufs=6,
                                   name=f"pt{h}_{j}_{tkb}")
                    nc.scalar.activation(out=pt, in_=sps,
                                         func=mybir.ActivationFunctionType.Exp,
                                         scale=SCALE)
                    if tkb >= 4 * j:
                        # causal: zero pt where tq_free < tk_part + 128*r
                        nc.gpsimd.affine_select(
                            out=pt, in_=pt,
                            compare_op=mybir.AluOpType.is_ge,
                            fill=0.0,
                            base=-(128 * (tkb - 4 * j)),
                            pattern=[[1, 512]],
                            channel_multiplier=-1,
                        )
                    nc.tensor.matmul(out=otps, lhsT=v_sb[tkb], rhs=pt,
                                     start=(tkb == 0), stop=(tkb == ntk - 1))
                    if tkb == 0:
                        nc.vector.tensor_copy(out=racc, in_=pt)
                    else:
                        nc.vector.tensor_add(out=racc, in0=racc, in1=pt)
                rsb = psum.tile([128, 512], F32, tag="acc", bufs=3,
                                name=f"rsb{h}_{j}")
                nc.tensor.matmul(out=rsb, lhsT=ones32, rhs=racc,
                                 start=True, stop=True)
                rinv = work.tile([128, 512], F32, tag="rinv", bufs=2,
                                 name=f"rinv{h}_{j}")
                nc.vector.reciprocal_approx_fast(rinv, rsb)
                nc.vector.tensor_mul(out=oT[h][:, sl], in0=otps, in1=rinv)

            # --- output projection for the 4 t-tiles of slice j ---
            for sub in range(4):
                tt = 4 * j + sub
                ostg = work.tile([128, D], F32, tag="ostg", bufs=2,
                                 name=f"ostg{tt}")
                for n in range(4):
                    ops = psum.tile([128, 512], F32, tag="op", bufs=2,
                                    name=f"ops{tt}_{n}")
                    for h in range(HPC):
                        nc.tensor.matmul(
                            out=ops,
                            lhsT=oT[h][:, tt * 128:(tt + 1) * 128],
                            rhs=wo_sb[h][:, n * 512:(n + 1) * 512],
                            start=(h == 0), stop=(h == HPC - 1))
                    nc.vector.tensor_copy(out=ostg[:, n * 512:(n + 1) * 512],
                                          in_=ops)
                nc.sync.dma_start(out=part[tt * 128:(tt + 1) * 128, :],
                                  in_=ostg)

    nc.compile()
    return nc


def _get_nc():
    if "nc" not in _CACHE:
        _CACHE["nc"] = _build_nc()
    return _CACHE["nc"]


def _bf16(a):
    return np.ascontiguousarray(a.astype(ml_dtypes.bfloat16))


def kernel(x, Wq, bq, Wk, bk, Wv, bv, Wo, bo, **kw):
    x = np.asarray(x, dtype=np.float32)
    Wq = np.asarray(Wq, dtype=np.float32)
    Wk = np.asarray(Wk, dtype=np.float32)
    Wv = np.asarray(Wv, dtype=np.float32)
    Wo = np.asarray(Wo, dtype=np.float32)
    bq = np.asarray(bq, dtype=np.float32)
    bk = np.asarray(bk, dtype=np.float32)
    bv = np.asarray(bv, dtype=np.float32)
    bo = np.asarray(bo, dtype=np.float32)

    nc = _get_nc()
    xt_b = [_bf16(x[b].T) for b in range(B)]
    in_maps = []
    for c in range(NCORES):
        b = c // 4
        q = c % 4
        hs = q * HPC * DH          # column start in Wq / row start in Wo
        kv = q // 2
        bq_m = np.ascontiguousarray(
            bq[hs:hs + HPC * DH].reshape(HPC, DH).T)          # [128, 4]
        bk_m = np.ascontiguousarray(
            bk[kv * DH:(kv + 1) * DH].reshape(DH, 1))         # [128, 1]
        bv_m = np.ascontiguousarray(
            bv[kv * DH:(kv + 1) * DH].reshape(DH, 1))         # [128, 1]
        in_maps.append({
            "xt": xt_b[b],
            "wq": _bf16(Wq[:, hs:hs + HPC * DH]),
            "wk": _bf16(Wk[:, kv * DH:(kv + 1) * DH]),
            "wv": _bf16(Wv[:, kv * DH:(kv + 1) * DH]),
            "wo": _bf16(Wo[hs:hs + HPC * DH, :]),
            "bqm": bq_m,
            "bkm": bk_m,
            "bvm": bv_m,
        })

    res = run_bass_kernel_spmd(nc, in_maps, list(range(NCORES)),
                               **kw.get("_run_kwargs", {}))
    if kw.get("_return_res"):
        return res
    parts = [res.results[c]["part"] for c in range(NCORES)]
    out = np.empty((B, T, D), dtype=np.float32)
    for b in range(B):
        acc = parts[4 * b].astype(np.float32).copy()
        for q in range(1, 4):
            acc += parts[4 * b + q]
        out[b] = acc + bo[None, :]
    return out



# revision 9
# speedup vs baseline: 1.2547x; 1.2547x over previous
"""GQA kernel for Trainium2, 8 NeuronCores.

Problem: B=2, T=2048, D=2048, 16 query heads / 2 KV heads, d_head=128, causal.

Sharding: core c -> batch b = c//4, head-quarter q = c%4 (query heads
4q..4q+3, kv head q//2). Each core computes its 4 heads' attention and a
partial output projection (its Wo rows); host sums the 4 partials per batch
and adds bo.

Host marshalling: weights and x are pre-cast to bf16 and pre-tiled so each
logical input lands with ONE multi-dim DMA per priority chunk:
  xt [128, 16, 2048] = x[b].T tiled as (p, kb, t)
  wq [128, 16, 512], wk/wv [128, 16, 128]  (p, kb, cols)
  wo [128, 4, 2048]                        (p=dh, h, n)
DMA priority order (two queues, sync+gpsimd, halves of each chunk):
biases, wk, wv, x slice0, wq, wo, x slices 1-3 -- so PE starts ~9us in and
never starves.

On-core dataflow (bf16 matmuls, fp32 PSUM):
per 512-wide t-slice j: K/V/Q projections (ACT epilogues w/ bias);
V PE-transposed to natural layout; attention per head h over tk blocks,
one [128,512] PSUM bank per S^T block (exp on ACT, never bank-crossing),
causal diagonal blocks column-trimmed (S/exp/PV/racc only touch the valid
tq range, stored at column 0; 128x128 affine_select masks on gpsimd only
for the true diagonal), PV accumulated into otps; softmax denominator:
bf16 pair-tree + fp32 racc on DVE, bf16 cast, one ones-matmul partition
reduction per (h,j); normalize (reciprocal+mul) on DVE.
Output projection for slice j is cut into 4-matmul units used as PE filler
inside slice j+1's attention rounds (keeps PE fed while ACT exp runs);
PSUM->SBUF staging copies split DVE/ACT; output DMA on the scalar queue.
"""

import numpy as np
import ml_dtypes
from contextlib import ExitStack

import concourse.bass as bass
from concourse import bacc
import concourse.mybir as mybir
import concourse.tile as tile
from concourse.bass_utils import run_bass_kernel_spmd
from concourse.masks import make_identity

F32 = mybir.dt.float32
F32R = mybir.dt.float32r
BF16 = mybir.dt.bfloat16

D = 2048
T = 2048
DH = 128
B = 2
HPC = 4            # query heads per core
NCORES = 8
SCALE = 1.0 / float(np.sqrt(128.0))

_CACHE = {}


def _build_nc():
    nc = bacc.Bacc("TRN2", target_bir_lowering=False, debug=False,
                   num_devices=NCORES)

    xt = nc.dram_tensor("xt", [128, 16, T], BF16, kind="ExternalInput")
    wq = nc.dram_tensor("wq", [128, 16, HPC * DH], BF16, kind="ExternalInput")
    wk = nc.dram_tensor("wk", [128, 16, DH], BF16, kind="ExternalInput")
    wv = nc.dram_tensor("wv", [128, 16, DH], BF16, kind="ExternalInput")
    wo = nc.dram_tensor("wo", [128, HPC, D], BF16, kind="ExternalInput")
    bqm = nc.dram_tensor("bqm", [DH, HPC], F32, kind="ExternalInput")
    bkm = nc.dram_tensor("bkm", [DH, 1], F32, kind="ExternalInput")
    bvm = nc.dram_tensor("bvm", [DH, 1], F32, kind="ExternalInput")
    part = nc.dram_tensor("part", [T, D], F32, kind="ExternalOutput")

    with ExitStack() as ctx:
        tc = ctx.enter_context(tile.TileContext(nc))
        persist = ctx.enter_context(tc.tile_pool(name="persist", bufs=1))
        work = ctx.enter_context(tc.tile_pool(name="work", bufs=2))
        psum = ctx.enter_context(tc.tile_pool(name="psum", bufs=2, space="PSUM"))

        # ---- constants ----
        ones32 = persist.tile([128, 128], BF16, tag="ones32", name="ones32")
        nc.vector.memset(ones32, 1.0)
        ident = persist.tile([128, 128], BF16, tag="ident", name="ident")
        make_identity(nc, ident)
        # warm the ACT exp table-set (~2.7us) during the initial DMA wait
        warm = persist.tile([128, 1], F32, tag="warm", name="warm")
        nc.scalar.activation(out=warm, in_=ident[:, 0:1],
                             func=mybir.ActivationFunctionType.Exp)

        bq_sb = persist.tile([DH, HPC], F32, tag="bq", name="bq_sb")
        bk_sb = persist.tile([DH, 1], F32, tag="bk", name="bk_sb")
        bv_sb = persist.tile([DH, 1], F32, tag="bv", name="bv_sb")

        # ---- persistent input tiles ----
        x_all = persist.tile([128, 16, T], BF16, tag="x_all", name="x_all")
        wq_sb = persist.tile([128, 16, HPC * DH], BF16, tag="wq", name="wq_sb")
        wk_sb = persist.tile([128, 16, DH], BF16, tag="wk", name="wk_sb")
        wv_sb = persist.tile([128, 16, DH], BF16, tag="wv", name="wv_sb")
        wo_sb = persist.tile([128, HPC, D], BF16, tag="wo", name="wo_sb")

        # ---- input DMAs in consumption order on two queues ----
        nc.sync.dma_start(out=bq_sb, in_=bqm[:, :])
        nc.sync.dma_start(out=bk_sb, in_=bkm[:, :])
        nc.sync.dma_start(out=bv_sb, in_=bvm[:, :])
        nc.sync.dma_start(out=wk_sb[:, 0:8, :], in_=wk[:, 0:8, :])
        nc.gpsimd.dma_start(out=wk_sb[:, 8:16, :], in_=wk[:, 8:16, :])
        nc.sync.dma_start(out=wv_sb[:, 0:8, :], in_=wv[:, 0:8, :])
        nc.gpsimd.dma_start(out=wv_sb[:, 8:16, :], in_=wv[:, 8:16, :])
        nc.sync.dma_start(out=x_all[:, 0:8, 0:512], in_=xt[:, 0:8, 0:512])
        nc.gpsimd.dma_start(out=x_all[:, 8:16, 0:512], in_=xt[:, 8:16, 0:512])
        nc.sync.dma_start(out=wq_sb[:, :, 0:256], in_=wq[:, :, 0:256])
        nc.gpsimd.dma_start(out=wq_sb[:, :, 256:512], in_=wq[:, :, 256:512])
        nc.sync.dma_start(out=wo_sb[:, 0:2, :], in_=wo[:, 0:2, :])
        nc.gpsimd.dma_start(out=wo_sb[:, 2:4, :], in_=wo[:, 2:4, :])
        for js in range(1, 4):
            sl = slice(js * 512, (js + 1) * 512)
            nc.sync.dma_start(out=x_all[:, 0:8, sl], in_=xt[:, 0:8, sl])
            nc.gpsimd.dma_start(out=x_all[:, 8:16, sl], in_=xt[:, 8:16, sl])

        # ---- persistent activations ----
        kT = persist.tile([128, T], BF16, tag="kT", name="kT")
        v_sb = [persist.tile([128, DH], BF16, tag=f"v{t}", name=f"v{t}")
                for t in range(16)]
        # per-slice q and o (o double-buffered: outproj(j) runs during j+1)
        qT = [persist.tile([128, 512], BF16, tag=f"qT{h}", name=f"qT{h}")
              for h in range(HPC)]
        oT = [[persist.tile([128, 512], BF16, tag=f"oT{d}_{h}",
                            name=f"oT{d}_{h}")
               for h in range(HPC)] for d in range(2)]

        # ---------- filler machinery ----------
        # Each filler unit is a closure emitting a few PE matmuls (+ epilogue
        # ops on other engines).  Units are popped inside attention pair
        # rounds to keep PE busy while ACT computes exp.
        fillers = []

        def pop_fillers(k):
            for _ in range(min(k, len(fillers))):
                fillers.pop(0)()

        def drain_fillers():
            while fillers:
                fillers.pop(0)()

        # ---------- projection helpers ----------
        def qproj_quarter(j, h, qps, kq):
            def emit():
                for kb in range(4 * kq, 4 * kq + 4):
                    nc.tensor.matmul(out=qps,
                                     lhsT=wq_sb[:, kb, h * 128:(h + 1) * 128],
                                     rhs=x_all[:, kb, j * 512:(j + 1) * 512],
                                     start=(kb == 0), stop=(kb == 15))
                if kq == 3:
                    nc.scalar.activation(out=qT[h], in_=qps,
                                         func=mybir.ActivationFunctionType.Identity,
                                         bias=bq_sb[:, h:h + 1], scale=1.0)
            return emit

        def emit_qproj(j, h):
            qps = psum.tile([128, 512], F32, tag="acc", bufs=2,
                            name=f"qps{j}_{h}")
            for kq in range(4):
                qproj_quarter(j, h, qps, kq)()

        def emit_kvproj(j):
            sl = slice(j * 512, (j + 1) * 512)
            kps = psum.tile([128, 512], F32, tag="acc", bufs=2, name=f"kps{j}")
            for kb in range(16):
                nc.tensor.matmul(out=kps, lhsT=wk_sb[:, kb, :],
                                 rhs=x_all[:, kb, sl],
                                 start=(kb == 0), stop=(kb == 15))
            nc.scalar.activation(out=kT[:, sl], in_=kps,
                                 func=mybir.ActivationFunctionType.Identity,
                                 bias=bk_sb[:, 0:1], scale=1.0)
            vps = psum.tile([128, 512], F32, tag="acc", bufs=2, name=f"vps{j}")
            for kb in range(16):
                nc.tensor.matmul(out=vps, lhsT=wv_sb[:, kb, :],
                                 rhs=x_all[:, kb, sl],
                                 start=(kb == 0), stop=(kb == 15))
            vt_sb = work.tile([128, 512], BF16, tag="vt", bufs=2,
                              name=f"vt{j}")
            nc.scalar.activation(out=vt_sb, in_=vps,
                                 func=mybir.ActivationFunctionType.Identity,
                                 bias=bv_sb[:, 0:1], scale=1.0)
            vtp = psum.tile([128, 512], BF16, tag="op", bufs=2, name=f"vtp{j}")
            for sub in range(4):
                nc.tensor.transpose(vtp[:, sub * 128:(sub + 1) * 128],
                                    vt_sb[:, sub * 128:(sub + 1) * 128],
                                    ident)
            for sub in range(4):
                nc.vector.tensor_copy(out=v_sb[4 * j + sub],
                                      in_=vtp[:, sub * 128:(sub + 1) * 128])

        # ---------- output projection units (filler fodder) ----------
        _ostg = {}

        def outproj_unit(j, tt, n, last_tt):
            # 4 matmuls (contract heads) + staging copy (+ DMA when ready)
            def emit():
                if tt not in _ostg:
                    _ostg[tt] = work.tile([128, D], F32, tag="ostg", bufs=2,
                                          name=f"ostg{tt}")
                ostg = _ostg[tt]
                ops = psum.tile([128, 512], F32, tag="op", bufs=2,
                                name=f"ops{tt}_{n}")
                sub = tt % 4
                for h in range(HPC):
                    nc.tensor.matmul(
                        out=ops,
                        lhsT=oT[j % 2][h][:, sub * 128:(sub + 1) * 128],
                        rhs=wo_sb[:, h, n * 512:(n + 1) * 512],
                        start=(h == 0), stop=(h == HPC - 1))
                if n % 2 == 0:
                    nc.vector.tensor_copy(
                        out=ostg[:, n * 512:(n + 1) * 512], in_=ops)
                else:
                    nc.scalar.copy(out=ostg[:, n * 512:(n + 1) * 512],
                                   in_=ops)
                if last_tt:
                    # per-unit DMA to shorten the kernel tail
                    nc.scalar.dma_start(
                        out=part[tt * 128:(tt + 1) * 128,
                                 n * 512:(n + 1) * 512],
                        in_=ostg[:, n * 512:(n + 1) * 512])
                elif n == 3:
                    nc.scalar.dma_start(
                        out=part[tt * 128:(tt + 1) * 128, :], in_=ostg)
            return emit

        def queue_outproj(j):
            for sub in range(4):
                tt = 4 * j + sub
                for n in range(4):
                    fillers.append(
                        outproj_unit(j, tt, n, last_tt=(tt == 15)))

        # ---------- attention ----------
        def emit_attention_head(j, h, fill_rate):
            """Attention for head h over tq-slice j, tk blocks 0..4j+3.
            One [128,512] PSUM bank per S^T block (exp never crosses banks,
            matmul outs always bank-aligned); the 4 diagonal blocks are
            column-trimmed: block r computes only tq in [128r, 512), stored
            at column 0 of its tile.  Softmax denominator accumulates via a
            bf16 pair tree on DVE."""
            otps = psum.tile([128, 512], F32, tag="acc", bufs=2,
                             name=f"otps{h}_{j}")
            racc = work.tile([128, 512], F32, tag="racc", bufs=2,
                             name=f"racc{h}_{j}")

            def sp_block(tkb, w_off):
                """S^T block for tk tile tkb covering tq [w_off:512), then
                exp -> pt[:, 0:512-w_off]."""
                sps = psum.tile([128, 512], F32, tag="sps", bufs=4,
                                name=f"sps{h}_{j}_{tkb}")
                w = 512 - w_off
                nc.tensor.matmul(
                    out=sps[:, 0:w],
                    lhsT=kT[:, tkb * 128:(tkb + 1) * 128],
                    rhs=qT[h][:, w_off:512],
                    start=True, stop=True)
                pt = work.tile([128, 512], BF16, tag="pt", bufs=4,
                               name=f"pt{h}_{j}_{tkb}")
                nc.scalar.activation(out=pt[:, 0:w], in_=sps[:, 0:w],
                                     func=mybir.ActivationFunctionType.Exp,
                                     scale=SCALE)
                return pt

            def pv_mm(tkb, pt_ap, o_off, start, stop):
                nc.tensor.matmul(out=otps[:, o_off:512], lhsT=v_sb[tkb],
                                 rhs=pt_ap, start=start, stop=stop,
                                 skip_group_check=True)

            # --- non-diagonal blocks, racc'd in bf16 pairs ---
            for p in range(2 * j):
                pt0 = sp_block(2 * p, 0)
                pv_mm(2 * p, pt0, 0, start=(p == 0), stop=False)
                pt1 = sp_block(2 * p + 1, 0)
                pv_mm(2 * p + 1, pt1, 0, start=False, stop=False)
                if p == 0:
                    nc.vector.tensor_add(out=racc, in0=pt0, in1=pt1)
                else:
                    pp = work.tile([128, 512], BF16, tag="ppair", bufs=2,
                                   name=f"pp{h}_{j}_{p}")
                    nc.vector.tensor_add(out=pp, in0=pt0, in1=pt1)
                    nc.vector.tensor_add(out=racc, in0=racc, in1=pp)
                pop_fillers(fill_rate)

            # --- diagonal blocks r=0..3, column-trimmed ---
            base = 4 * j
            for r in range(4):
                w_off = 128 * r
                w = 512 - w_off
                pt = sp_block(base + r, w_off)
                nc.gpsimd.affine_select(
                    out=pt[:, 0:128], in_=pt[:, 0:128],
                    compare_op=mybir.AluOpType.is_ge,
                    fill=0.0, base=0,
                    pattern=[[1, 128]], channel_multiplier=-1)
                pv_mm(base + r, pt[:, 0:w], w_off,
                      start=(j == 0 and r == 0), stop=(r == 3))
                if j == 0 and r == 0:
                    nc.vector.tensor_copy(out=racc, in_=pt)
                else:
                    nc.vector.tensor_add(out=racc[:, w_off:512],
                                         in0=racc[:, w_off:512],
                                         in1=pt[:, 0:w])
                pop_fillers(fill_rate)

            # --- denominator: bf16 ones-matmul partition reduction ---
            racc16 = work.tile([128, 512], BF16, tag="racc16", bufs=2,
                               name=f"racc16{h}_{j}")
            nc.vector.tensor_copy(out=racc16, in_=racc)
            rsb = psum.tile([128, 512], F32, tag="op", bufs=2,
                            name=f"rsb{h}_{j}")
            nc.tensor.matmul(out=rsb, lhsT=ones32, rhs=racc16,
                             start=True, stop=True)
            rinv = work.tile([128, 512], F32, tag="rinv", bufs=2,
                             name=f"rinv{h}_{j}")
            nc.vector.reciprocal_approx_fast(rinv, rsb)
            nc.vector.tensor_mul(out=oT[j % 2][h], in0=otps, in1=rinv)

        # ---------- main schedule ----------
        for j in range(4):
            emit_kvproj(j)
            if j == 0:
                emit_qproj(0, 0)
                emit_qproj(0, 1)
                qps2 = psum.tile([128, 512], F32, tag="acc", bufs=2,
                                 name="qps0_2f")
                fillers.extend(qproj_quarter(0, 2, qps2, kq)
                               for kq in range(4))
                emit_attention_head(0, 0, fill_rate=2)
                drain_fillers()
                qps3 = psum.tile([128, 512], F32, tag="acc", bufs=2,
                                 name="qps0_3f")
                fillers.extend(qproj_quarter(0, 3, qps3, kq)
                               for kq in range(4))
                emit_attention_head(0, 1, fill_rate=2)
                drain_fillers()
                emit_attention_head(0, 2, fill_rate=0)
                emit_attention_head(0, 3, fill_rate=0)
            else:
                for h in range(HPC):
                    emit_qproj(j, h)
                rate = 1
                for h in range(HPC):
                    emit_attention_head(j, h, fill_rate=rate)
            queue_outproj(j)
            if j > 0:
                # keep at most one slice's worth of units pending
                while len(fillers) > 16:
                    fillers.pop(0)()
        drain_fillers()

    nc.compile()
    return nc


def _get_nc():
    if "nc" not in _CACHE:
        _CACHE["nc"] = _build_nc()
    return _CACHE["nc"]


def _bf16(a):
    return np.ascontiguousarray(a.astype(ml_dtypes.bfloat16))


def _tile16(a):
    # [2048, C] -> [128, 16, C]   (rows kb*128+p -> [p, kb, :])
    c = a.shape[1]
    return np.ascontiguousarray(
        a.reshape(16, 128, c).transpose(1, 0, 2))


def kernel(x, Wq, bq, Wk, bk, Wv, bv, Wo, bo, **kw):
    x = np.asarray(x, dtype=np.float32)
    Wq = np.asarray(Wq, dtype=np.float32)
    Wk = np.asarray(Wk, dtype=np.float32)
    Wv = np.asarray(Wv, dtype=np.float32)
    Wo = np.asarray(Wo, dtype=np.float32)
    bq = np.asarray(bq, dtype=np.float32)
    bk = np.asarray(bk, dtype=np.float32)
    bv = np.asarray(bv, dtype=np.float32)
    bo = np.asarray(bo, dtype=np.float32)

    nc = _get_nc()
    xt_b = [_tile16(_bf16(x[b].T)) for b in range(B)]
    in_maps = []
    for c in range(NCORES):
        b = c // 4
        q = c % 4
        hs = q * HPC * DH          # column start in Wq / row start in Wo
        kv = q // 2
        bq_m = np.ascontiguousarray(
            bq[hs:hs + HPC * DH].reshape(HPC, DH).T)          # [128, 4]
        bk_m = np.ascontiguousarray(
            bk[kv * DH:(kv + 1) * DH].reshape(DH, 1))         # [128, 1]
        bv_m = np.ascontiguousarray(
            bv[kv * DH:(kv + 1) * DH].reshape(DH, 1))         # [128, 1]
        in_maps.append({
            "xt": xt_b[b],
            "wq": _tile16(_bf16(Wq[:, hs:hs + HPC * DH])),
            "wk": _tile16(_bf16(Wk[:, kv * DH:(kv + 1) * DH])),
            "wv": _tile16(_bf16(Wv[:, kv * DH:(kv + 1) * DH])),
            "wo": np.ascontiguousarray(
                _bf16(Wo[hs:hs + HPC * DH, :]).reshape(HPC, 128, D)
                .transpose(1, 0, 2)),
            "bqm": bq_m,
            "bkm": bk_m,
            "bvm": bv_m,
        })

    res = run_bass_kernel_spmd(nc, in_maps, list(range(NCORES)),
                               **kw.get("_run_kwargs", {}))
    if kw.get("_return_res"):
        return res
    parts = [res.results[c]["part"] for c in range(NCORES)]
    out = np.empty((B, T, D), dtype=np.float32)
    for b in range(B):
        acc = parts[4 * b].astype(np.float32).copy()
        for q in range(1, 4):
            acc += parts[4 * b + q]
        out[b] = acc + bo[None, :]
    return out
